# revision 41
# baseline (speedup 1.0000x reference)
"""Trainium2 Bass kernel for the O2O classification head (GNN message passing).

Strategy (v2: polynomial-gelu kernel trick)
-------------------------------------------
The reference edge tensor is rank-structured per feature dim:
    pre_gelu[b,i,j,d] = a[b,i,d] - q[b,j,d]
with a = (feats@W_in + pos@W_pos + b_in + b_pos)@W_e1 + b_e1 - b_out@W_e1
and  q = (feats@W_out + pos@W_pos)@W_e1.

The edge score  s(i,j) = sum_d w_d * gelu(a_id - q_jd) + b_e2  is replaced by
a degree-2 polynomial P ~= gelu fitted per batch on the realized range of x
(x in ~[-0.6, 0.6]; gelu has no x^3 term, so D=2 already gives final rel err
~2e-5 against a 2e-2 gate).  Binomial expansion turns s into (D+1) inner
products over 128 dims:
    s(i,j) = sum_k  U^k_i . V^k_j,   U^k = w * a^k,  V^k = sum_l gamma_kl q^l
so the whole N^2 x 128 gelu+dot becomes 3 bf16 PE matmuls per 128-j block per
256-i half (1 cycle/row) -- no per-pair gelu at all; block1's two i-halves
accumulate in separate PSUM tiles so the first half's masked max overlaps the
second half's matmuls.  W_in@W_e1 / W_out@W_e1 / W_pos@W_e1
are folded on the host so a and q come straight out of two matmul chains.

Masking: nodes are host-sorted by (cls desc, id desc) so suppress[i,j] != 0
iff rank_i < rank_j; the triangle is an int16 iota compare folded into the
mask STT chain (tensor_mask_reduce crashes the exec unit on real TRN2, and
walrus rejects TensorScalarPtr on Pool -- both flags kept off).  The angle
window |ang_i - ang_j| < 0.5 runs on the otherwise-idle Scalar engine as a
Sign pair -- Sign(ang_i-(ang_j-0.5)) * Sign((ang_j+0.5)-ang_i) == 1 exactly
on the open window (both-negative impossible) -- leaving DVE a bf16 multiply
+ clamp.  The angle row broadcast is a hi/lo bf16 split accumulated in PSUM
(error ~5e-6, fed to ACT straight from PSUM).  Masked entries multiply to 0
and the plain reduce_max reproduces the reference's max-over-zeros
semantics.

Sharding: 2 cores per batch; core P takes sorted-rank j blocks [128P, 128P+128)
and [256+128P, 256+128P+128), so every core's block0 only needs i in [0,256)
(triangle) while block1 needs [0,512) -- identical SPMD shapes, 25% less work.
Dummy PE matmuls during the DMA window pre-ramp the HAM clock gate; the node
MLP tail runs in bf16 and sigmoid/threshold/unsort happen on the host.
"""

import math
import sys
import numpy as np

if "/opt/trn_rl_repo" not in sys.path:
    sys.path.insert(0, "/opt/trn_rl_repo")

import ml_dtypes

B, N = 4, 512
H_DIM, I_DIM = 64, 128
N_CORES = 8
NJ = 256             # j's per core
DEG = 2              # polynomial degree for gelu fit (gelu has no x^3 term)
F32 = np.float32
BF16 = ml_dtypes.bfloat16

IMG_W, IMG_H, CENTER_H = 800.0, 320.0, 160.0
NUM_OFFSETS = 72
CONF_THRES = 0.4

# ---- wpack (fp32 [128, WCOLS]) column layout ----
WC_BCLS = 0        # [64,1]
WC_BPOS = 1        # [128,1]
WC_BE1 = 2         # [128,1]
WC_WCOL = 3        # [128,1]
WC_BN1 = 4         # [64,1]
WC_BN2 = 5         # [64,1]
WC_BH = 6          # [1,1]
WC_ACOL = 7        # [128,2]  ang_j per block
WC_ACOLM = 9       # [128,2]  ang_j - 0.5
WC_REND = 11       # [128,2]  rank_j (f32) per block
WC_GAM = 13        # [128,16] poly coefficient columns
WCOLS = 29         # (identity lives in bpack as bf16)

# ---- bpack (bf16 [128, BCOLS]) column layout ----
BC_WCLS = 0        # [64,64]
BC_WINE1 = 64      # [64,128]  W_in @ W_e1
BC_WOUTE1 = 192    # [64,128]  W_out @ W_e1
BC_WPOSE1 = 320    # [2,128]   W_pos @ W_e1
BC_WN1 = 448       # [2,64]    W_n1 duplicated on partitions 0 and 1
BC_WN2 = 512       # [64,64]
BC_WHEAD = 576     # [64,1]
BC_IDENT = 577     # [128,128] bf16 identity (PE transpose)
BCOLS = 705

# gam column indices (D=2): 0: g00' (incl b_e2 fold), 1: g01, 2: g02,
# 4: g10, 5: g11, 7: g20

USE_GPSIMD_MASKS = False  # walrus: TensorScalarPtr not supported on Pool
USE_TMR = False      # tensor_mask_reduce crashes real HW (NRT_EXEC_UNIT_UNRECOVERABLE)
_PROGRAM = None

INPUT_SPECS_F32 = [("wpack", (128, WCOLS))]
INPUT_SPECS_BF16 = [
    ("ang2", (1, 1024)),     # hi/lo bf16 split of the fp32 angle row
    ("wbcp", (128, 512)),    # W_bcast[d,i] = w_d (k=0 channel rhs)
    ("bpack", (128, BCOLS)),
    ("bft", (64, 512)),      # feats.T (i-side, sorted)
    ("bftj", (64, 256)),     # feats.T (j-side, core-local)
    ("posT", (2, 512)),      # pos.T (i-side)
    ("posTj", (2, 256)),     # pos.T (j-side)
]


def _build_program(num_devices=N_CORES):
    import contextlib
    import concourse.bass as bass  # noqa: F401
    import concourse.tile as tile
    from concourse import bacc, mybir

    f32 = mybir.dt.float32
    bf16 = mybir.dt.bfloat16
    AF = mybir.ActivationFunctionType
    OP = mybir.AluOpType

    nc = bacc.Bacc("TRN2", target_bir_lowering=False, debug=False,
                   num_devices=num_devices)

    dram = {}
    for nm, shape in INPUT_SPECS_F32:
        dram[nm] = nc.declare_dram_parameter(nm, list(shape), f32, isOutput=False)
    for nm, shape in INPUT_SPECS_BF16:
        dram[nm] = nc.declare_dram_parameter(nm, list(shape), bf16, isOutput=False)
    y = nc.declare_dram_parameter("y", [1, 256], f32, isOutput=True)

    with tile.TileContext(nc) as tc:
        with contextlib.ExitStack() as ctx:
            const = ctx.enter_context(tc.tile_pool(name="const", bufs=1))
            work = ctx.enter_context(tc.tile_pool(name="work", bufs=2))
            pps = ctx.enter_context(tc.tile_pool(name="pps", bufs=4, space="PSUM"))
            spsum = ctx.enter_context(tc.tile_pool(name="spsum", bufs=2, space="PSUM"))
            tpsum = ctx.enter_context(tc.tile_pool(name="tpsum", bufs=2, space="PSUM"))

            # ---- table-load trigger (hide the ~2.7us ACT table load) ----
            junk = const.tile([1, 2], f32, name="junk", tag="junk")
            nc.vector.memset(junk[:], 0.0)
            nc.scalar.activation(junk[0:1, 1:2], junk[0:1, 0:1], AF.Relu)
            nc.scalar.activation(junk[0:1, 1:2], junk[0:1, 0:1], AF.Sign)

            # ---- input DMAs (spread across engine queues) ----
            wp = const.tile([128, WCOLS], f32, name="wp", tag="wp")
            bp = const.tile([128, BCOLS], bf16, name="bp", tag="bp")
            bft = const.tile([64, 512], bf16, name="bft", tag="bft")
            bftj = const.tile([64, 256], bf16, name="bftj", tag="bftj")
            posT = const.tile([2, 512], bf16, name="posT", tag="posT")
            posTj = const.tile([2, 256], bf16, name="posTj", tag="posTj")
            ang2 = const.tile([1, 1024], bf16, name="ang2", tag="ang2")
            nc.sync.dma_start(out=ang2[:], in_=dram["ang2"][:])
            # critical matmul weights (cols 0:448) land first; the tail-MLP
            # weights + identity ride later (needed only at ~19us).
            nc.scalar.dma_start(out=bp[:, 0:448], in_=dram["bpack"][:, 0:448])
            nc.scalar.dma_start(out=wp[:], in_=dram["wpack"][:])
            nc.scalar.dma_start(out=bp[:, 448:BCOLS],
                                in_=dram["bpack"][:, 448:BCOLS])
            nc.gpsimd.dma_start(out=bft[:], in_=dram["bft"][:])
            nc.gpsimd.dma_start(out=bftj[:], in_=dram["bftj"][:])
            wbc = const.tile([128, 512], bf16, name="wbc", tag="wbc")
            nc.gpsimd.dma_start(out=wbc[:], in_=dram["wbcp"][:])
            nc.sync.dma_start(out=posTj[:], in_=dram["posTj"][:])
            nc.sync.dma_start(out=posT[:], in_=dram["posT"][:])
            jm = const.tile([128, 512], bf16, name="jm", tag="jm")
            nc.vector.memset(jm[:], 0.5)
            ones_f = const.tile([1, 128], bf16, name="ones_f", tag="ones_f")
            nc.vector.memset(ones_f[:], 1.0)
            if not USE_TMR:
                iotab = const.tile([128, 512], mybir.dt.int16, name="iotab",
                                   tag="iotab")
                nc.gpsimd.iota(iotab[:], pattern=[[1, 512]], base=0,
                               channel_multiplier=0)

            # PE warmup: dummy matmuls keep the PE busy through the DMA
            # window so the HAM clock-gate ramps before the real work.
            ps_w = pps.tile([128, 512], f32, name="ps_w", tag="ps")
            for wi in range(3):
                nc.tensor.matmul(ps_w[:], jm[:, 0:128], jm[:],
                                 start=(wi == 0), stop=(wi == 2),
                                 skip_group_check=True)

            # named slices of the packs
            bcls = wp[0:64, WC_BCLS:WC_BCLS + 1]
            bpos = wp[0:128, WC_BPOS:WC_BPOS + 1]
            be1 = wp[0:128, WC_BE1:WC_BE1 + 1]
            wcol = wp[0:128, WC_WCOL:WC_WCOL + 1]
            bn1 = wp[0:64, WC_BN1:WC_BN1 + 1]
            bn2 = wp[0:64, WC_BN2:WC_BN2 + 1]
            bh = wp[0:1, WC_BH:WC_BH + 1]
            ident = bp[0:128, BC_IDENT:BC_IDENT + 128]

            def gam(i):
                return wp[0:128, WC_GAM + i:WC_GAM + i + 1]

            wcls = bp[0:64, BC_WCLS:BC_WCLS + 64]
            wine1 = bp[0:64, BC_WINE1:BC_WINE1 + 128]
            woute1 = bp[0:64, BC_WOUTE1:BC_WOUTE1 + 128]
            wpose1 = bp[0:2, BC_WPOSE1:BC_WPOSE1 + 128]
            wn2 = bp[0:64, BC_WN2:BC_WN2 + 64]
            whead = bp[0:64, BC_WHEAD:BC_WHEAD + 1]
            bfT = bft[0:64, 0:512]
            bfTj = bftj[0:64, 0:256]

            # angle row broadcast: hi/lo bf16 split accumulated in PSUM
            # reconstructs the fp32 row to ~5e-6 at bf16 matmul speed.
            ps_ang = pps.tile([128, 512], f32, name="ps_ang", tag="ps")
            nc.tensor.matmul(ps_ang[:], ones_f[:], ang2[0:1, 0:512],
                             start=True, stop=False)
            nc.tensor.matmul(ps_ang[:], ones_f[:], ang2[0:1, 512:1024],
                             start=False, stop=True)
            angb = ps_ang

            # q-side first (depends only on bftj/posTj, not on feats)
            ps_q = pps.tile([128, 256], f32, name="ps_q", tag="ps")
            nc.tensor.matmul(ps_q[:], woute1, bfTj[:], start=True, stop=False)
            nc.tensor.matmul(ps_q[:], wpose1, posTj[:], start=False, stop=True)
            qbf = const.tile([128, 256], bf16, name="qbf", tag="qbf")
            nc.scalar.copy(qbf[:], ps_q[:])

            # ---- masks (DVE; independent of the weight chains) ----
            # angle window via Sign pair on the (otherwise idle) ACT engine:
            # s1 = Sign(ang_i - (ang_j-0.5)), s2 = Sign((ang_j+0.5) - ang_i);
            # s1*s2 == 1 exactly on the open window (both -1 impossible),
            # then clamp to {0,1} and fold the rank triangle.
            ILEN = [256, 512]     # i-prefix per block (triangle-aware)
            masks = {}
            for bi in range(2):
                L = ILEN[bi]
                negacolm = wp[0:128, WC_ACOL + bi:WC_ACOL + bi + 1]
                acolp = wp[0:128, WC_ACOLM + bi:WC_ACOLM + bi + 1]
                rend = wp[0:128, WC_REND + bi:WC_REND + bi + 1]
                s1 = work.tile([128, L], bf16, name=f"s1m{bi}", tag=f"s1m{bi}")
                nc.scalar.activation(s1[:], angb[:, 0:L], AF.Sign,
                                     bias=negacolm)
                s2 = work.tile([128, L], bf16, name=f"s2m{bi}", tag=f"s2m{bi}")
                nc.scalar.activation(s2[:], angb[:, 0:L], AF.Sign,
                                     bias=acolp, scale=-1.0)
                p = work.tile([128, L], bf16, name=f"pm{bi}", tag=f"pm{bi}")
                nc.vector.tensor_tensor(p[:], s1[:], s2[:], OP.mult)
                nc.vector.tensor_scalar_max(p[:], p[:], 0.0)
                m2 = const.tile([128, L], bf16, name=f"m2{bi}", tag=f"m2{bi}")
                nc.vector.scalar_tensor_tensor(m2[:], iotab[:, 0:L],
                                               rend, p[:], OP.is_lt,
                                               OP.logical_and)
                masks[bi] = m2

            # ---- preprocessing (folded weights: W_in@W_e1 etc) ----
            ps_f = pps.tile([64, 512], f32, name="ps_f", tag="ps")
            nc.tensor.matmul(ps_f[:], wcls, bfT, start=True, stop=True)
            featsT = const.tile([64, 512], bf16, name="featsT", tag="featsT")
            nc.scalar.activation(featsT[:], ps_f[:], AF.Relu, bias=bcls)

            ps_p = pps.tile([128, 512], f32, name="ps_p", tag="ps")
            nc.tensor.matmul(ps_p[:], wine1, featsT[:], start=True, stop=False)
            nc.tensor.matmul(ps_p[:], wpose1, posT[:], start=False, stop=True)
            abf = const.tile([128, 512], bf16, name="abf", tag="abf")
            nc.scalar.activation(abf[:], ps_p[:], AF.Identity, bias=be1)

            # ---- feature maps (DVE) ----
            q2 = const.tile([128, 256], bf16, name="q2", tag="q2")
            nc.vector.tensor_tensor(q2[:], qbf[:], qbf[:], OP.mult)

            U = [None] * (DEG + 1)
            for k in range(1, DEG + 1):
                U[k] = const.tile([128, 512], bf16, name=f"U{k}", tag=f"U{k}")
            nc.vector.tensor_scalar_mul(U[1][:], abf[:], wcol)
            nc.vector.tensor_tensor(U[2][:], U[1][:], abf[:], OP.mult)

            V = [None] * (DEG + 1)
            for k in range(DEG + 1):
                V[k] = const.tile([128, 256], bf16, name=f"V{k}", tag=f"V{k}")
            # V1 = g10 + g11 q
            nc.vector.tensor_scalar(V[1][:], qbf[:], gam(5), gam(4),
                                    OP.mult, OP.add)
            # V2 = g20
            nc.vector.tensor_scalar(V[2][:], qbf[:], 0.0, gam(7),
                                    OP.mult, OP.add)
            # V0 = g00' + g01 q + g02 q^2
            nc.vector.tensor_scalar(V[0][:], qbf[:], gam(1), gam(0),
                                    OP.mult, OP.add)
            nc.vector.scalar_tensor_tensor(V[0][:], q2[:], gam(2), V[0][:],
                                           OP.mult, OP.add)

            # ---- main matmuls + masked max per 128-j block ----
            # block0: one 256-col group; block1: two 256-col half groups so
            # the first half's masked-mult+max overlaps the second half's
            # matmuls and the final reduce is half-width.
            nmb = const.tile([128, 2], bf16, name="nmb", tag="nmb")
            nm2 = const.tile([128, 2], bf16, name="nm2", tag="nm2")
            halves = [(1, slice(0, 256), nm2[:, 0:1]),
                      (1, slice(256, 512), nm2[:, 1:2]),
                      (0, slice(0, 256), nmb[:, 0:1])]
            for hi, (bi, isl, acc) in enumerate(halves):
                jsl = slice(128 * bi, 128 * bi + 128)
                S = spsum.tile([128, 256], f32, name=f"S{hi}", tag="S")
                for k in range(1, DEG + 1):
                    nc.tensor.matmul(S[:], V[k][:, jsl], U[k][:, isl],
                                     start=(k == 1), stop=False)
                nc.tensor.matmul(S[:], V[0][:, jsl], wbc[:, isl],
                                 start=False, stop=True)
                msk = work.tile([128, 256], bf16, name=f"msk{hi}",
                                tag=f"msk{hi}")
                nc.vector.tensor_tensor(msk[:], S[:], masks[bi][:, isl],
                                        OP.mult)
                nc.vector.reduce_max(acc, msk[:], axis=mybir.AxisListType.X)
            nc.vector.reduce_max(nmb[:, 1:2], nm2[:],
                                 axis=mybir.AxisListType.X)

            # ---- node MLP tail ----
            wn1 = bp[0:1, BC_WN1:BC_WN1 + 64]
            nmrows = {}
            for bi in range(2):
                ps_t = tpsum.tile([1, 128], bf16, name=f"ps_t{bi}", tag="pt")
                nc.tensor.transpose(ps_t[:], nmb[:, bi:bi + 1], ident)
                nmrow = work.tile([1, 128], bf16, name=f"nmrow{bi}",
                                  tag=f"nmrow{bi}")
                nc.vector.tensor_copy(nmrow[:], ps_t[:])
                nmrows[bi] = nmrow

            ps_n1 = tpsum.tile([64, 256], f32, name="ps_n1", tag="pt")
            s1 = work.tile([64, 256], bf16, name="s1", tag="s1")
            for bi in range(2):
                nc.tensor.matmul(ps_n1[:, 128 * bi:128 * bi + 128], wn1,
                                 nmrows[bi][:], start=True, stop=True)
            nc.vector.tensor_scalar(s1[:], ps_n1[:], bn1, 0.0,
                                    OP.add, OP.max)
            ps_n2 = tpsum.tile([64, 256], f32, name="ps_n2", tag="pt")
            nc.tensor.matmul(ps_n2[:], wn2, s1[:], start=True, stop=True)
            s2 = work.tile([64, 256], bf16, name="s2", tag="s2")
            nc.vector.tensor_scalar(s2[:], ps_n2[:], bn2, 0.0,
                                    OP.add, OP.max)
            ps_L = tpsum.tile([1, 256], f32, name="ps_L", tag="pt")
            nc.tensor.matmul(ps_L[:], whead, s2[:], start=True, stop=True)
            out_t = work.tile([1, 256], f32, name="out_t", tag="out_t")
            nc.vector.tensor_copy(out_t[:], ps_L[:])
            nc.sync.dma_start(out=y[:], in_=out_t[:])

    nc.compile()
    return nc


def _get_program():
    global _PROGRAM
    if _PROGRAM is None:
        _PROGRAM = _build_program()
    return _PROGRAM


def _pos_emb(e0, e1):
    """float32 mirror of the reference _get_sample_point (one batch, sorted)."""
    angle = (e0 * F32(np.pi)).astype(F32)
    rho = (e1 * F32(IMG_W)).astype(F32)
    lin = np.linspace(0.0, 1.0 - 1e-5, NUM_OFFSETS, dtype=F32)
    yk = (F32(CENTER_H) - lin * F32(IMG_H)).astype(F32)[:2]
    tan = np.tan(angle, dtype=F32)
    roc = (rho / np.cos(angle, dtype=F32)).astype(F32)
    x = (-tan[:, None] * yk[None, :] + roc[:, None]).astype(F32)
    return (x / F32(IMG_W)).astype(F32)          # [n, 2]


def _gelu_np(x):
    v = np.vectorize(lambda t: 0.5 * t * (1.0 + math.erf(t / math.sqrt(2.0))))
    return v(np.asarray(x, np.float64))


def kernel(**inputs):
    bf = np.asarray(inputs["batch_features"], dtype=F32)      # [B,N,64]
    cls = np.asarray(inputs["cls_pred"], dtype=F32)           # [B,N]
    aid = np.asarray(inputs["anchor_id"])                     # [B,N] int32
    emb = np.asarray(inputs["anchor_embeddings"], dtype=F32)  # [B,N,2]

    w = {k: np.asarray(inputs[k], dtype=F32) for k in
         ("W_cls", "b_cls", "W_pos", "b_pos", "W_in", "b_in", "W_out", "b_out",
          "W_e1", "b_e1", "W_e2", "b_e2", "W_n1", "b_n1", "W_n2", "b_n2",
          "W_head", "b_head")}
    bpos_eff = (w["b_in"] + w["b_pos"]).astype(F32)
    be1_eff = (w["b_e1"] - w["b_out"] @ w["W_e1"]
               + bpos_eff @ w["W_e1"]).astype(F32)
    wine1 = (w["W_in"] @ w["W_e1"]).astype(F32)               # [64,128]
    woute1 = (w["W_out"] @ w["W_e1"]).astype(F32)             # [64,128]
    wpose1 = (w["W_pos"] @ w["W_e1"]).astype(F32)             # [2,128]
    we2 = w["W_e2"][:, 0]                                     # [128]

    nc = _get_program()
    from concourse.bass_utils import run_bass_kernel_spmd

    # ---- static packs (shared across cores) ----
    wpack0 = np.zeros((128, WCOLS), dtype=F32)
    wpack0[0:64, WC_BCLS] = w["b_cls"]
    wpack0[0:128, WC_BPOS] = bpos_eff
    wpack0[0:128, WC_BE1] = be1_eff
    wpack0[0:128, WC_WCOL] = we2
    wpack0[0:64, WC_BN1] = w["b_n1"]
    wpack0[0:64, WC_BN2] = w["b_n2"]
    wpack0[0, WC_BH] = w["b_head"][0]


    bpack = np.zeros((128, BCOLS), dtype=BF16)
    bpack[0:64, BC_WCLS:BC_WCLS + 64] = w["W_cls"].astype(BF16)
    bpack[0:64, BC_WINE1:BC_WINE1 + 128] = wine1.astype(BF16)
    bpack[0:64, BC_WOUTE1:BC_WOUTE1 + 128] = woute1.astype(BF16)
    bpack[0:2, BC_WPOSE1:BC_WPOSE1 + 128] = wpose1.astype(BF16)
    bpack[0:1, BC_WN1:BC_WN1 + 64] = w["W_n1"].astype(BF16)
    bpack[1:2, BC_WN1:BC_WN1 + 64] = w["W_n1"].astype(BF16)
    bpack[0:64, BC_WN2:BC_WN2 + 64] = w["W_n2"].astype(BF16)
    bpack[0:64, BC_WHEAD] = w["W_head"][:, 0].astype(BF16)
    bpack[0:128, BC_IDENT:BC_IDENT + 128] = np.eye(128, dtype=BF16)

    wbcp = np.repeat(we2[:, None], 512, axis=1).astype(BF16)
    sw2 = float(np.sum(we2.astype(np.float64) ** 2))
    sw2 = sw2 if sw2 > 1e-20 else 1e-20

    in_maps = []
    perms = []
    cls_sorted = []
    jidx_all = []
    for b in range(B):
        perm = np.lexsort((-aid[b].astype(np.int64), -cls[b]))
        perms.append(perm)
        bf_s = bf[b][perm]                    # [N, 64]
        cls_s = cls[b][perm]
        cls_sorted.append(cls_s)
        e0_s = emb[b][perm, 0]
        e1_s = emb[b][perm, 1]
        ang_s = (e0_s * F32(np.pi)).astype(F32)
        pos_s = _pos_emb(e0_s, e1_s)          # [N, 2]

        # host fp32 mirror of preprocessing for the adaptive poly fit range
        feats = np.maximum(bf_s @ w["W_cls"] + w["b_cls"], 0).astype(F32)
        pe = (pos_s @ w["W_pos"]).astype(F32)
        A = (feats @ w["W_in"] + pe + bpos_eff).astype(F32)
        Cm = (feats @ w["W_out"] + pe).astype(F32)
        a_h = (A @ w["W_e1"] + be1_eff).astype(F32)
        q_h = (Cm @ w["W_e1"]).astype(F32)
        lo = float((a_h.min(0) - q_h.max(0)).min()) - 0.25
        hi = float((a_h.max(0) - q_h.min(0)).max()) + 0.25
        g = np.cos(np.linspace(0.0, np.pi, 2001))
        grid = (lo + hi) / 2 + (hi - lo) / 2 * g
        cs = np.polynomial.polynomial.polyfit(grid, _gelu_np(grid), DEG)

        # gamma_{k,l} = c_{k+l} * C(k+l,k) * (-1)^l
        def gm(k, l):
            return F32(cs[k + l] * math.comb(k + l, k) * ((-1.0) ** l))

        gcols = np.zeros((128, 16), dtype=F32)
        gcols[:, 0] = gm(0, 0) + F32(w["b_e2"][0]) * we2 / F32(sw2)
        for l in range(1, DEG + 1):
            gcols[:, l] = gm(0, l)
        gcols[:, 4] = gm(1, 0); gcols[:, 5] = gm(1, 1)
        gcols[:, 7] = gm(2, 0)

        bft_all = bf_s.T.astype(BF16)
        posT_all = pos_s.T.astype(BF16)

        for P in range(2):
            wpk = wpack0.copy()
            blks = []
            for bi in range(2):
                lo = 128 * P + 256 * bi
                blk = slice(lo, lo + 128)
                blks.append(blk)
                wpk[0:128, WC_ACOL + bi] = -(ang_s[blk] - F32(0.5))
                wpk[0:128, WC_ACOLM + bi] = ang_s[blk] + F32(0.5)
                wpk[0:128, WC_REND + bi] = np.arange(lo, lo + 128, dtype=F32)
            jidx = np.concatenate([np.arange(b.start, b.stop) for b in blks])
            wpk[0:128, WC_GAM:WC_GAM + 16] = gcols

            jidx_all.append(jidx)
            hi = ang_s.astype(BF16)
            lo = (ang_s - hi.astype(F32)).astype(BF16)
            ang2h = np.concatenate([hi, lo])[None, :]
            in_maps.append({
                "wpack": wpk,
                "ang2": ang2h,
                "wbcp": wbcp,
                "bpack": bpack,
                "bft": bft_all,
                "bftj": np.ascontiguousarray(bf_s[jidx].T).astype(BF16),
                "posT": posT_all,
                "posTj": np.ascontiguousarray(pos_s[jidx].T).astype(BF16),
            })

    res = run_bass_kernel_spmd(nc, in_maps, list(range(N_CORES)))

    out = np.zeros((B, N), dtype=F32)
    for ci in range(N_CORES):
        b = ci // 2
        jidx = jidx_all[ci]
        logits = np.asarray(res.results[ci]["y"][0], dtype=np.float64)
        logits = logits + float(w["b_head"][0])
        probs = (1.0 / (1.0 + np.exp(-logits))).astype(F32)
        probs = np.where(cls_sorted[b][jidx] < F32(CONF_THRES), F32(0.0),
                         probs)
        out[b, perms[b][jidx]] = probs
    return out
